# revision 16
# baseline (speedup 1.0000x reference)
"""BinHD Hamming-distance kernel for 8 Trainium2 NeuronCores.

dist[n, c] = sum_d xor(samples[n, d], classes_hv[c, d])
           = s_sum[n] + c_sum[c] - 2 * (samples @ classes_hv.T)[n, c]

Strategy (data-parallel over samples):
  - shard samples row-wise across 8 cores (1024 rows each); replicate classes.
  - per core: a [1024 x 9984] x [9984 x 1000] GEMM on the TensorEngine in
    fp8e4m3 with perf_mode=DoubleRow (2 MACs/cell/cycle). Inputs are {0,1} and
    {0,-2} -> fp8 is exact; PSUM accumulates fp32 and |sums| < 2^24 -> the
    result is bit-exact vs the fp32 reference.
  - classes are pre-scaled by -2 so PSUM directly holds -2*cross; the epilogue
    is a single DVE add of a host-precomputed bias plane
    bias[n, c] = s_sum[n] + c_sum[c] - 2 * samples[n, 9984:] @ classes[c, 9984:]
    (the K remainder 10000 = 39*256 + 16 is folded into the bias on the host,
    saving a full 16-wide super-tile of N=512 matmuls on the PE).

DoubleRow layout: each matmul contracts K=256 via 3D APs [p, i, free] with
k = 256*t + 128*i + p (planar i-major packing in SBUF, validated on HW).

DMA: operands are host-packed per-partition-contiguous; transfers are staged
small-first (64KB..1MB) so the first matmul starts ~1-2us into the kernel
while steady-state DMAs run at ~8KB/partition descriptors (>400 GB/s).
"""

import sys

if "/opt/trn_rl_repo" not in sys.path:
    sys.path.insert(0, "/opt/trn_rl_repo")

import numpy as np
import ml_dtypes

N, D, C = 8192, 10000, 1000
N_CORES = 8
P = 128
TT = 39                  # k-super-tiles of 256 on the PE (covers 9984 of D)
K_MM = TT * 2 * P        # 9984
C_PAD = 1008             # classes padded 1000 -> 1008 (512 + 496 psum chunks)
NQ = 2
QW = [512, 496]          # psum chunk widths (496 = smallest mult-of-16 >= 488)
ST_B = 2 * (QW[0] + QW[1])   # bt elements per supertile per partition (2016)
M_SH = N // N_CORES      # 1024 sample rows per core
MT = M_SH // P           # 8 m-tiles per core
PAIRS = MT // 2          # m-tiles processed in pairs


def _staged_sizes(total, ramp, steady):
    sizes = []
    rem = total
    for r in ramp:
        if rem <= 0:
            break
        s = min(r, rem)
        sizes.append(s)
        rem -= s
    while rem > 0:
        s = min(steady, rem)
        sizes.append(s)
        rem -= s
    return sizes


BT_SIZES = _staged_sizes(TT, [1, 1, 2], 4)    # supertiles per bt DMA group
BT_STARTS = np.cumsum([0] + BT_SIZES).tolist()
# at DMA groups: ramp small-first only on pair 0 (kernel start); steady 8 after
AT_SIZES_RAMP = _staged_sizes(TT, [1, 1, 2, 4], 8)
AT_SIZES_STEADY = _staged_sizes(TT, [], 8)
AT_PLAN = [
    (sizes, np.cumsum([0] + sizes).tolist())
    for sizes in (AT_SIZES_RAMP, AT_SIZES_STEADY)
]

F8 = ml_dtypes.float8_e4m3

_compiled = None


def _build():
    import concourse.mybir as mybir
    from concourse import bacc
    from concourse.tile import TileContext

    nc = bacc.Bacc("TRN2", target_bir_lowering=False, debug=False)
    f8 = mybir.dt.float8e4
    f32 = mybir.dt.float32
    DR = mybir.MatmulPerfMode.DoubleRow

    # at[pair]: [p, (t mi i m)] samplesT, per-partition contiguous
    at_d = nc.declare_dram_parameter("at", [PAIRS, P, TT * 512], f8, isOutput=False)
    # bt: [p, (t q i n)] (-2*classes).T, per-partition contiguous
    bt_d = nc.declare_dram_parameter("bt", [P, TT * ST_B], f8, isOutput=False)
    bias_d = nc.declare_dram_parameter("bias", [MT, P, C_PAD], f32, isOutput=False)
    out_d = nc.declare_dram_parameter("out", [MT, P, C_PAD], f32, isOutput=True)

    with TileContext(nc) as tc:
        with (
            tc.tile_pool(name="btp", bufs=1) as btp,
            tc.tile_pool(name="atp", bufs=4) as atp,
            tc.tile_pool(name="pp", bufs=2, space="PSUM") as pp,
            tc.tile_pool(name="op", bufs=3) as op,
            tc.tile_pool(name="bp", bufs=3) as bp,
        ):
            # classes stay resident in SBUF (39 x 2KB/partition); loads are
            # interleaved with the first pair's k-loop, staged small-first.
            btgs = [None] * len(BT_SIZES)

            for pair in range(PAIRS):
                AT_SIZES, AT_STARTS = AT_PLAN[0 if pair == 0 else 1]
                ps = [
                    pp.tile([P, QW[j % 2]], f32, tag=f"ps{j}", name=f"ps{j}")
                    for j in range(4)
                ]
                # bias planes are only consumed in the epilogue; issue their
                # DMAs early so they are never on the tail critical path --
                # but not at t=0 of pair 0, where they would delay the
                # kernel-start operand loads on the same HWDGE queue.
                bias_ts = [None, None]
                bias_issue_t = TT - 1 if pair == 0 else 0
                ag = None
                for t in range(TT):
                    g_b = int(np.searchsorted(BT_STARTS, t, side="right")) - 1
                    g_a = int(np.searchsorted(AT_STARTS, t, side="right")) - 1
                    j_b = t - BT_STARTS[g_b]
                    j_a = t - AT_STARTS[g_a]
                    if t == bias_issue_t:
                        for mi in range(2):
                            bias_t = bp.tile(
                                [P, C_PAD], f32, tag="bias_t", name="bias_t"
                            )
                            nc.sync.dma_start(
                                out=bias_t, in_=bias_d[2 * pair + mi]
                            )
                            bias_ts[mi] = bias_t
                    if pair == 0 and j_b == 0:
                        btg = btp.tile(
                            [P, BT_SIZES[g_b] * ST_B], f8,
                            tag=f"btg{g_b}", name=f"btg{g_b}",
                        )
                        nc.sync.dma_start(
                            out=btg,
                            in_=bt_d[
                                :, BT_STARTS[g_b] * ST_B:BT_STARTS[g_b + 1] * ST_B
                            ],
                        )
                        btgs[g_b] = btg
                    if j_a == 0:
                        ag = atp.tile(
                            [P, AT_SIZES[g_a] * 512], f8, tag="ag", name="ag"
                        )
                        nc.sync.dma_start(
                            out=ag,
                            in_=at_d[
                                pair, :, AT_STARTS[g_a] * 512:AT_STARTS[g_a + 1] * 512
                            ],
                        )
                    btg = btgs[g_b]
                    for mi in range(2):
                        lhs3 = ag[
                            :, (j_a * 2 + mi) * 256:(j_a * 2 + mi + 1) * 256
                        ].rearrange("p (i m) -> p i m", i=2)
                        for q in range(NQ):
                            qb = j_b * ST_B + q * 2 * QW[0]
                            rhs3 = btg[
                                :, qb:qb + 2 * QW[q]
                            ].rearrange("p (i n) -> p i n", i=2)
                            nc.tensor.matmul(
                                ps[2 * mi + q], lhs3, rhs3,
                                start=(t == 0), stop=(t == TT - 1),
                                perf_mode=DR,
                            )
                for mi in range(2):
                    m = 2 * pair + mi
                    bias_t = bias_ts[mi]
                    o = op.tile([P, C_PAD], f32)
                    nc.vector.tensor_add(o[:, 0:512], ps[2 * mi][:], bias_t[:, 0:512])
                    nc.sync.dma_start(out=out_d[m, :, 0:512], in_=o[:, 0:512])
                    nc.vector.tensor_add(
                        o[:, 512:C_PAD], ps[2 * mi + 1][:], bias_t[:, 512:C_PAD]
                    )
                    nc.sync.dma_start(out=out_d[m, :, 512:C_PAD], in_=o[:, 512:C_PAD])

    nc.compile()
    return nc


def _prep_inputs(samples: np.ndarray, classes_hv: np.ndarray):
    """Host-side shard + layout prep. All values stay exactly representable."""
    samples = np.ascontiguousarray(samples, dtype=np.float32)
    classes_hv = np.ascontiguousarray(classes_hv, dtype=np.float32)

    s_sum = samples.sum(axis=1, dtype=np.float32)        # [N], ints <= D
    c_sum = classes_hv.sum(axis=1, dtype=np.float32)     # [C]
    c_pad = np.zeros(C_PAD, np.float32)
    c_pad[:C] = c_sum
    bias_full = s_sum[:, None] + c_pad[None, :]          # [N, C_PAD] f32
    # K remainder (d >= 9984) folded into the bias plane (exact int math)
    bias_full[:, :C] += (-2.0 * samples[:, K_MM:]) @ classes_hv[:, K_MM:].T

    # bt: (-2*classes).T [K_MM, C_PAD]; k = 256t + 128i + p -> [p, (t q i n)]
    B8 = np.zeros((K_MM, C_PAD), F8)
    B8[:, :C] = (-2.0 * classes_hv[:, :K_MM]).astype(F8).T
    b0 = (
        B8[:, :QW[0]].reshape(TT, 2, P, QW[0])
        .transpose(2, 0, 1, 3).reshape(P, TT, 2 * QW[0])
    )
    b1 = (
        B8[:, QW[0]:].reshape(TT, 2, P, QW[1])
        .transpose(2, 0, 1, 3).reshape(P, TT, 2 * QW[1])
    )
    bt_host = np.ascontiguousarray(
        np.concatenate([b0, b1], axis=2).reshape(P, TT * ST_B)
    )

    in_maps = []
    for c in range(N_CORES):
        rows = slice(c * M_SH, (c + 1) * M_SH)
        A8 = samples[rows, :K_MM].astype(F8).T           # [K_MM, 1024]
        # [k, m] -> [pair, p, (t mi i m)]
        at_c = np.ascontiguousarray(
            A8.reshape(TT, 2, P, PAIRS, 2, P)            # [t, i, p, pair, mi, m]
            .transpose(3, 2, 0, 4, 1, 5)                 # [pair, p, t, mi, i, m]
            .reshape(PAIRS, P, TT * 512)
        )
        bias_c = np.ascontiguousarray(bias_full[rows].reshape(MT, P, C_PAD))
        in_maps.append({"at": at_c, "bt": bt_host, "bias": bias_c})
    return in_maps


def _run(inputs: dict, trace: bool = False, **spmd_kwargs):
    from concourse.bass_utils import run_bass_kernel_spmd

    global _compiled
    if _compiled is None:
        _compiled = _build()

    in_maps = _prep_inputs(inputs["samples"], inputs["classes_hv"])
    res = run_bass_kernel_spmd(
        _compiled, in_maps, list(range(N_CORES)), trace=trace, **spmd_kwargs
    )
    parts = [
        res.results[c]["out"].reshape(M_SH, C_PAD)[:, :C] for c in range(N_CORES)
    ]
    out = np.concatenate(parts, axis=0).astype(np.float32)
    return out, res


def kernel(samples: np.ndarray, classes_hv: np.ndarray) -> np.ndarray:
    out, _ = _run({"samples": samples, "classes_hv": classes_hv})
    return out


# revision 29
# speedup vs baseline: 1.0582x; 1.0582x over previous
"""BinHD Hamming-distance kernel for 8 Trainium2 NeuronCores.

dist[n, c] = sum_d xor(samples[n, d], classes_hv[c, d])
           = s_sum[n] + c_sum[c] - 2 * (samples @ classes_hv.T)[n, c]

Strategy (data-parallel over samples):
  - shard samples row-wise across 8 cores (1024 rows each); replicate classes.
  - per core: a [1024 x 9984] x [9984 x 1000] GEMM on the TensorEngine in
    fp8e4m3 with perf_mode=DoubleRow (2 MACs/cell/cycle). Inputs are {0,1} and
    {0,-2} -> fp8 is exact; PSUM accumulates fp32 and |sums| < 2^24 -> the
    result is bit-exact vs the fp32 reference.
  - classes are pre-scaled by -2 so PSUM directly holds -2*cross; the epilogue
    is a single DVE add of a host-precomputed bias plane
    bias[n, c] = s_sum[n] + c_sum[c] - 2 * samples[n, 9984:] @ classes[c, 9984:]
    (the K remainder 10000 = 39*256 + 16 is folded into the bias on the host,
    saving a full super-tile of matmuls on the PE).

DoubleRow layout: each matmul contracts K=256 via 3D APs [p, i, free] with
k = 256*t + 128*i + p (planar i-major packing in SBUF, validated on HW).

m-tiles are processed in k-passes of [3, 3, 2] tiles (6/6/4 PSUM banks,
explicitly bank-assigned): the first pass's compute window is 1.5x a pair's,
which keeps the one-time 9.6MB classes load off the PE critical path (with
2-tile groups the first window demands ~378 GB/s -- over the ~358 HBM limit).
DMAs are host-packed per-partition-contiguous and staged small-first so the
first matmul starts early while steady transfers stay >=256KB.
"""

import sys

if "/opt/trn_rl_repo" not in sys.path:
    sys.path.insert(0, "/opt/trn_rl_repo")

import numpy as np
import ml_dtypes

N, D, C = 8192, 10000, 1000
N_CORES = 8
P = 128
TT = 39                  # k-super-tiles of 256 on the PE (covers 9984 of D)
K_MM = TT * 2 * P        # 9984
C_PAD = 1008             # classes padded 1000 -> 1008 (512 + 496 psum chunks)
NQ = 2
QSTRIDE = [512, 496]     # SBUF i-plane strides (DoubleRow: stride % 16 == 0)
QW = [512, 488]          # streamed widths; q1 streams 488 of its 496 plane
ST_B = 2 * (QSTRIDE[0] + QSTRIDE[1])   # bt elements per supertile/partition
M_SH = N // N_CORES      # 1024 sample rows per core
MT = M_SH // P           # 8 m-tiles per core

# m-tile groups per k-pass and their PSUM bank assignments (8 banks total).
# Consecutive groups overlap: a group's first banks are ones the previous
# group did not use, so its matmuls start while the previous epilogue drains.
M_GROUPS = [[0, 1, 2, 3], [4, 5, 6], [7]]
BANKS = [
    [0, 1, 2, 3, 4, 5, 6, 7],  # group 0 (all banks; 4 m-tiles halve the
                               # early bt-demand per compute-second)
    [0, 1, 2, 3, 4, 5],        # group 1 (0,1 freed first by g0's epilogue)
    [6, 7],                    # group 2
]


def _staged_sizes(total, ramp, steady):
    sizes = []
    rem = total
    for r in ramp:
        if rem <= 0:
            break
        s = min(r, rem)
        sizes.append(s)
        rem -= s
    while rem > 0:
        s = min(steady, rem)
        sizes.append(s)
        rem -= s
    return sizes


BT_SIZES = _staged_sizes(TT, [1, 1, 2], 4)    # supertiles per bt DMA group
BT_STARTS = np.cumsum([0] + BT_SIZES).tolist()
# at DMA groups per m-tile: ramp small-first in pass 0, steady 8 after
AT_SIZES_RAMP = _staged_sizes(TT, [2, 2, 4], 8)
AT_SIZES_STEADY = _staged_sizes(TT, [], 8)
AT_PLAN = [
    (sizes, np.cumsum([0] + sizes).tolist())
    for sizes in (AT_SIZES_RAMP, AT_SIZES_STEADY)
]

F8 = ml_dtypes.float8_e4m3

_compiled = None


def _build():
    import concourse.mybir as mybir
    from concourse import bacc
    from concourse.tile import TileContext

    nc = bacc.Bacc("TRN2", target_bir_lowering=False, debug=False)
    f8 = mybir.dt.float8e4
    f32 = mybir.dt.float32
    DR = mybir.MatmulPerfMode.DoubleRow

    # at[m]: [p, (t i mcol)] samplesT for m-tile m, per-partition contiguous
    at_d = nc.declare_dram_parameter("at", [MT, P, TT * 256], f8, isOutput=False)
    # bt: [p, (t q i n)] (-2*classes).T, per-partition contiguous
    bt_d = nc.declare_dram_parameter("bt", [P, TT * ST_B], f8, isOutput=False)
    bias_d = nc.declare_dram_parameter("bias", [MT, P, C_PAD], f32, isOutput=False)
    out_d = nc.declare_dram_parameter("out", [MT, P, C_PAD], f32, isOutput=True)

    with TileContext(nc) as tc:
        with (
            tc.tile_pool(name="btp", bufs=1) as btp,
            tc.tile_pool(name="atp", bufs=6) as atp,
            tc.tile_pool(name="pp", bufs=1, space="PSUM") as pp,
            tc.tile_pool(name="op", bufs=3) as op,
            tc.tile_pool(name="bp", bufs=4) as bp,
        ):
            # classes stay resident in SBUF (39 x ~2KB/partition); loads are
            # interleaved with the first pass's k-loop, staged small-first.
            btgs = [None] * len(BT_SIZES)
            next_ags = None

            for gi, mgroup in enumerate(M_GROUPS):
                AT_SIZES, AT_STARTS = AT_PLAN[0 if gi == 0 else 1]
                nm = len(mgroup)
                ps = [
                    [
                        pp.tile(
                            [P, QW[q]], f32,
                            tag=f"bank{BANKS[gi][2 * li + q]}",
                            name=f"ps_g{gi}_m{li}_q{q}",
                        )
                        for q in range(NQ)
                    ]
                    for li in range(nm)
                ]
                bias_ts = [None] * nm
                # bias rides the GpSimd SWDGE queue, so it never contends
                # with the sync-FIFO operand streams -- only with HBM. Issue
                # pass 0's at mid-loop (past the cold-DMA crunch) so all
                # bias tiles land before the epilogue needs them; SWDGE's
                # ~1us/DMA emission would otherwise push them past the
                # k-loop end and stall the PSUM bank-release staircase.
                bias_issue_t = TT // 2 if gi == 0 else 2
                ags = [None] * nm
                for t in range(TT):
                    g_b = int(np.searchsorted(BT_STARTS, t, side="right")) - 1
                    g_a = int(np.searchsorted(AT_STARTS, t, side="right")) - 1
                    j_b = t - BT_STARTS[g_b]
                    j_a = t - AT_STARTS[g_a]
                    if t == bias_issue_t:
                        for li in range(nm):
                            bias_t = bp.tile(
                                [P, C_PAD], f32, tag="bias_t", name="bias_t"
                            )
                            nc.gpsimd.dma_start(
                                out=bias_t, in_=bias_d[mgroup[li]]
                            )
                            bias_ts[li] = bias_t
                    if gi == 0 and j_b == 0:
                        btg = btp.tile(
                            [P, BT_SIZES[g_b] * ST_B], f8,
                            tag=f"btg{g_b}", name=f"btg{g_b}",
                        )
                        nc.sync.dma_start(
                            out=btg,
                            in_=bt_d[
                                :, BT_STARTS[g_b] * ST_B:BT_STARTS[g_b + 1] * ST_B
                            ],
                        )
                        btgs[g_b] = btg
                    if j_a == 0:
                        for li in range(nm):
                            if t == 0 and next_ags is not None:
                                ags[li] = next_ags[li]
                                continue
                            ag = atp.tile(
                                [P, AT_SIZES[g_a] * 256], f8,
                                tag=f"ag{li}", name=f"ag{li}",
                            )
                            nc.sync.dma_start(
                                out=ag,
                                in_=at_d[
                                    mgroup[li], :,
                                    AT_STARTS[g_a] * 256:AT_STARTS[g_a + 1] * 256,
                                ],
                            )
                            ags[li] = ag
                    if t == TT - 3 and gi + 1 < len(M_GROUPS):
                        # software-pipeline: start the next group's first at
                        # tiles now so the pass boundary has no DMA bubble.
                        nxt = M_GROUPS[gi + 1]
                        nsz = AT_PLAN[1][0][0]
                        next_ags = []
                        for li in range(len(nxt)):
                            nag = atp.tile(
                                [P, nsz * 256], f8, tag=f"ag{li}", name=f"ag{li}"
                            )
                            nc.sync.dma_start(
                                out=nag, in_=at_d[nxt[li], :, 0:nsz * 256]
                            )
                            next_ags.append(nag)
                    btg = btgs[g_b]
                    for li in range(nm):
                        lhs3 = ags[li][
                            :, j_a * 256:(j_a + 1) * 256
                        ].rearrange("p (i m) -> p i m", i=2)
                        for q in range(NQ):
                            qb = j_b * ST_B + q * 2 * QSTRIDE[0]
                            rhs3 = btg[
                                :, qb:qb + 2 * QSTRIDE[q]
                            ].rearrange("p (i n) -> p i n", i=2)[:, :, 0:QW[q]]
                            nc.tensor.matmul(
                                ps[li][q], lhs3, rhs3,
                                start=(t == 0), stop=(t == TT - 1),
                                perf_mode=DR,
                            )
                for li in range(nm):
                    m = mgroup[li]
                    bias_t = bias_ts[li]
                    o = op.tile([P, C_PAD], f32)
                    nc.vector.tensor_add(o[:, 0:512], ps[li][0][:], bias_t[:, 0:512])
                    nc.sync.dma_start(out=out_d[m, :, 0:512], in_=o[:, 0:512])
                    nc.vector.tensor_add(
                        o[:, 512:C_PAD], ps[li][1][:], bias_t[:, 512:C_PAD]
                    )
                    nc.sync.dma_start(out=out_d[m, :, 512:C_PAD], in_=o[:, 512:C_PAD])

    nc.compile()
    return nc


def _prep_inputs(samples: np.ndarray, classes_hv: np.ndarray):
    """Host-side shard + layout prep. All values stay exactly representable."""
    samples = np.ascontiguousarray(samples, dtype=np.float32)
    classes_hv = np.ascontiguousarray(classes_hv, dtype=np.float32)

    s_sum = samples.sum(axis=1, dtype=np.float32)        # [N], ints <= D
    c_sum = classes_hv.sum(axis=1, dtype=np.float32)     # [C]
    c_pad = np.zeros(C_PAD, np.float32)
    c_pad[:C] = c_sum
    bias_full = s_sum[:, None] + c_pad[None, :]          # [N, C_PAD] f32
    # K remainder (d >= 9984) folded into the bias plane (exact int math)
    bias_full[:, :C] += (-2.0 * samples[:, K_MM:]) @ classes_hv[:, K_MM:].T

    # bt: (-2*classes).T [K_MM, C_PAD]; k = 256t + 128i + p -> [p, (t q i n)]
    B8 = np.zeros((K_MM, C_PAD), F8)
    B8[:, :C] = (-2.0 * classes_hv[:, :K_MM]).astype(F8).T
    b0 = (
        B8[:, :QSTRIDE[0]].reshape(TT, 2, P, QSTRIDE[0])
        .transpose(2, 0, 1, 3).reshape(P, TT, 2 * QSTRIDE[0])
    )
    b1 = (
        B8[:, QSTRIDE[0]:].reshape(TT, 2, P, QSTRIDE[1])
        .transpose(2, 0, 1, 3).reshape(P, TT, 2 * QSTRIDE[1])
    )
    bt_host = np.ascontiguousarray(
        np.concatenate([b0, b1], axis=2).reshape(P, TT * ST_B)
    )

    in_maps = []
    for c in range(N_CORES):
        rows = slice(c * M_SH, (c + 1) * M_SH)
        A8 = samples[rows, :K_MM].astype(F8).T           # [K_MM, 1024]
        # [k, m] -> [m-tile, p, (t i mcol)]
        at_c = np.ascontiguousarray(
            A8.reshape(TT, 2, P, MT, P)                  # [t, i, p, mt, m]
            .transpose(3, 2, 0, 1, 4)                    # [mt, p, t, i, m]
            .reshape(MT, P, TT * 256)
        )
        bias_c = np.ascontiguousarray(bias_full[rows].reshape(MT, P, C_PAD))
        in_maps.append({"at": at_c, "bt": bt_host, "bias": bias_c})
    return in_maps


def _run(inputs: dict, trace: bool = False, **spmd_kwargs):
    from concourse.bass_utils import run_bass_kernel_spmd

    global _compiled
    if _compiled is None:
        _compiled = _build()

    in_maps = _prep_inputs(inputs["samples"], inputs["classes_hv"])
    res = run_bass_kernel_spmd(
        _compiled, in_maps, list(range(N_CORES)), trace=trace, **spmd_kwargs
    )
    parts = [
        res.results[c]["out"].reshape(M_SH, C_PAD)[:, :C] for c in range(N_CORES)
    ]
    out = np.concatenate(parts, axis=0).astype(np.float32)
    return out, res


def kernel(samples: np.ndarray, classes_hv: np.ndarray) -> np.ndarray:
    out, _ = _run({"samples": samples, "classes_hv": classes_hv})
    return out
